# revision 1
# baseline (speedup 1.0000x reference)
"""Trainium2 kernel for nn_Model_15135464751445 (moe_routing).

Strategy: node-parallel sharding — the [B,N,L,d] token grid is split over
N across the 8 NeuronCores (25 nodes/core, attention is over L so shards
are independent). RevIN statistics are computed redundantly per core from
the full x_enc. The per-shard forward runs on-device; jax.lax.top_k is
replaced by an algebraically identical max/second-max top-2 gate (the
renormalized top-2 softmax weights equal softmax over the top-2 logits),
which avoids an internal compiler error in the stock top_k lowering.
"""
import numpy as np
import jax
import jax.numpy as jnp

B, L, N, C = 4, 96, 200, 1
D, H, DH, M = 128, 8, 16, 16
DFF, E, TOPK, LAYERS = 512, 4, 2, 3
PRED, TOD = 96, 288
EPS = 1e-5
NCORES = 8
NSH = N // NCORES  # 25 nodes per core


def _layernorm(x, g, b):
    m = jnp.mean(x, -1, keepdims=True)
    v = jnp.mean((x - m) ** 2, -1, keepdims=True)
    return (x - m) / jnp.sqrt(v + EPS) * g + b


def _performer(x, Wq, bq, Wk, bk, Wv, bv, Wo, bo, proj):
    sh = x.shape

    def heads(t):
        return jnp.swapaxes(t.reshape(sh[:-1] + (H, DH)), -2, -3)

    q = heads(x @ Wq + bq)
    k = heads(x @ Wk + bk)
    v = heads(x @ Wv + bv)
    dn = DH ** -0.25
    q = q * dn
    k = k * dn
    qd = q @ proj
    kd = k @ proj
    qn = 0.5 * jnp.sum(q * q, -1, keepdims=True)
    kn = 0.5 * jnp.sum(k * k, -1, keepdims=True)
    qp = jnp.exp(qd - qn - jnp.max(qd, -1, keepdims=True)) * (M ** -0.5) + 1e-6
    kp = jnp.exp(kd - kn - jnp.max(kd, axis=(-1, -2), keepdims=True)) * (M ** -0.5) + 1e-6
    kv = jnp.einsum('...lm,...ld->...md', kp, v)
    z = 1.0 / (jnp.einsum('...lm,...m->...l', qp, jnp.sum(kp, -2)) + 1e-6)
    o = jnp.einsum('...lm,...md->...ld', qp, kv) * z[..., None]
    o = jnp.swapaxes(o, -2, -3).reshape(sh)
    return o @ Wo + bo


def _moe(x, Wg, W1, b1, W2, b2):
    # top-2 softmax gate with renormalized weights, top_k-free formulation:
    # g_e = 1[l_e >= v2] * exp(l_e - v1) / (1 + exp(v2 - v1))
    logits = x @ Wg
    v1 = jnp.max(logits, -1, keepdims=True)
    masked = jnp.where(logits >= v1, -jnp.inf, logits)
    v2 = jnp.max(masked, -1, keepdims=True)
    sel = (logits >= v2).astype(x.dtype)
    g = sel * jnp.exp(logits - v1) / (1.0 + jnp.exp(v2 - v1))
    h = jax.nn.relu(jnp.einsum('...d,edf->...ef', x, W1) + b1)
    y = jnp.einsum('...ef,efd->...ed', h, W2) + b2
    return jnp.einsum('...ed,...e->...d', y, g)


def _block(x, Wq, bq, Wk, bk, Wv, bv, Wo, bo, proj, Wg, W1, b1, W2, b2,
           g2, be2, g3, be3):
    a = _performer(x, Wq, bq, Wk, bk, Wv, bv, Wo, bo, proj)
    x = _layernorm(x + a, g2, be2)
    f = _moe(x, Wg, W1, b1, W2, b2)
    return _layernorm(x + f, g3, be3)


def _forward_shard(n0, adaptive_sh, x_enc, x_mark, emb_W, emb_b, tod_table,
                   dow_table, sp_W, sp_Wq, sp_bq, sp_Wk, sp_bk, sp_Wv, sp_bv,
                   sp_Wo, sp_bo, sp_proj, sp_gate, sp_W1, sp_b1, sp_W2, sp_b2,
                   sp_ln2_g, sp_ln2_b, sp_ln3_g, sp_ln3_b, proj_W, proj_b,
                   LWq, Lbq, LWk, Lbk, LWv, Lbv, LWo, Lbo, Lproj, Lgate,
                   LW1, Lb1, LW2, Lb2, Lln2g, Lln2b, Lln3g, Lln3b,
                   mix_W, mix_b):
    # RevIN stats over the FULL x_enc (global over L and N)
    mean = jnp.mean(x_enc, axis=(1, 2), keepdims=True)
    std = jnp.sqrt(jnp.var(x_enc, axis=(1, 2), keepdims=True) + EPS)
    # shard slice over N
    x_sh = jax.lax.dynamic_slice_in_dim(x_enc, n0, NSH, axis=2)  # [B,L,NSH,C]
    xn = (x_sh - mean) / std
    xp = jnp.pad(xn, ((0, 0), (1, 1), (0, 0), (0, 0)))
    trend = (xp[:, :-2] + xp[:, 1:-1] + xp[:, 2:]) / 3.0
    season = xn - trend
    sp = jnp.swapaxes(xn, 1, 2).reshape(B, NSH, L * C) @ sp_W
    sp = _block(sp[:, :, None, :], sp_Wq, sp_bq, sp_Wk, sp_bk, sp_Wv, sp_bv,
                sp_Wo, sp_bo, sp_proj, sp_gate, sp_W1, sp_b1, sp_W2, sp_b2,
                sp_ln2_g, sp_ln2_b, sp_ln3_g, sp_ln3_b)
    tod_f = jnp.broadcast_to(x_mark[:, :, None, 6:7], (B, L, NSH, 1))
    dow_f = jnp.broadcast_to(x_mark[:, :, None, 2:3], (B, L, NSH, 1))
    te = jnp.swapaxes(jnp.concatenate([trend, tod_f, dow_f], -1) @ emb_W + emb_b, 1, 2)
    se = jnp.swapaxes(jnp.concatenate([season, tod_f, dow_f], -1) @ emb_W + emb_b, 1, 2)
    ad = jnp.broadcast_to(adaptive_sh.reshape(1, NSH, L, D), (B, NSH, L, D))
    tod_idx = (x_mark[..., 6] * TOD).astype(jnp.int32)
    tod_e = jnp.broadcast_to(tod_table[tod_idx][:, None], (B, NSH, L, D))
    dow_idx = x_mark[..., 2].astype(jnp.int32)
    dow_e = jnp.broadcast_to(dow_table[dow_idx][:, None], (B, NSH, L, D))
    spb = jnp.broadcast_to(sp, (B, NSH, L, D))
    xe = jnp.concatenate([te, se, ad, tod_e, dow_e, spb], -1) @ proj_W + proj_b
    for i in range(LAYERS):
        xe = _block(xe, LWq[i], Lbq[i], LWk[i], Lbk[i], LWv[i], Lbv[i],
                    LWo[i], Lbo[i], Lproj[i], Lgate[i], LW1[i], Lb1[i],
                    LW2[i], Lb2[i], Lln2g[i], Lln2b[i], Lln3g[i], Lln3b[i])
    out = xe.reshape(B, NSH, L * D) @ mix_W + mix_b
    out = jnp.swapaxes(out.reshape(B, NSH, PRED, C), 1, 2)  # [B,PRED,NSH,C]
    return out * std + mean


_WEIGHT_KEYS = ['x_enc', 'x_mark', 'emb_W', 'emb_b', 'tod_table', 'dow_table',
                'sp_W', 'sp_Wq', 'sp_bq', 'sp_Wk', 'sp_bk', 'sp_Wv', 'sp_bv',
                'sp_Wo', 'sp_bo', 'sp_proj', 'sp_gate', 'sp_W1', 'sp_b1',
                'sp_W2', 'sp_b2', 'sp_ln2_g', 'sp_ln2_b', 'sp_ln3_g',
                'sp_ln3_b', 'proj_W', 'proj_b', 'LWq', 'Lbq', 'LWk', 'Lbk',
                'LWv', 'Lbv', 'LWo', 'Lbo', 'Lproj', 'Lgate', 'LW1', 'Lb1',
                'LW2', 'Lb2', 'Lln2g', 'Lln2b', 'Lln3g', 'Lln3b',
                'mix_W', 'mix_b']

_pmapped = None


def _get_pmapped():
    global _pmapped
    if _pmapped is None:
        _pmapped = jax.pmap(
            _forward_shard,
            in_axes=(0, 0) + (None,) * len(_WEIGHT_KEYS),
            devices=jax.devices()[:NCORES],
        )
    return _pmapped


def kernel(**inputs):
    d = {k: np.asarray(v) for k, v in inputs.items()}
    # host-side sharding only: slice adaptive_table per core over N
    ad_full = d['adaptive_table'].reshape(N, L, D)
    ad_sh = np.stack([ad_full[c * NSH:(c + 1) * NSH] for c in range(NCORES)])
    n0 = np.arange(NCORES, dtype=np.int32) * NSH
    fn = _get_pmapped()
    args = [d[k] for k in _WEIGHT_KEYS]
    out_sh = fn(n0, ad_sh, *args)  # [8, B, PRED, NSH, C]
    out_sh = np.asarray(out_sh)
    out = np.concatenate([out_sh[c] for c in range(NCORES)], axis=2)
    return out.astype(np.float32)


if __name__ == '__main__':
    rng = np.random.default_rng(0)
    print(jax.devices())


# revision 2
# speedup vs baseline: 28.3539x; 28.3539x over previous
"""Trainium2 kernel for nn_Model_15135464751445 (moe_routing).

Strategy: node-parallel sharding — the [B,N,L,d] token grid is split over
N across the 8 NeuronCores (25 nodes/core, attention is over L so shards
are independent). RevIN statistics are computed redundantly per core from
the full x_enc. The per-shard forward runs on-device; jax.lax.top_k is
replaced by an algebraically identical max/second-max top-2 gate (the
renormalized top-2 softmax weights equal softmax over the top-2 logits),
which avoids an internal compiler error in the stock top_k lowering.
"""
import numpy as np
import jax
import jax.numpy as jnp

B, L, N, C = 4, 96, 200, 1
D, H, DH, M = 128, 8, 16, 16
DFF, E, TOPK, LAYERS = 512, 4, 2, 3
PRED, TOD = 96, 288
EPS = 1e-5
NCORES = 8
NSH = N // NCORES  # 25 nodes per core


def _layernorm(x, g, b):
    m = jnp.mean(x, -1, keepdims=True)
    v = jnp.mean((x - m) ** 2, -1, keepdims=True)
    return (x - m) / jnp.sqrt(v + EPS) * g + b


def _performer(x, Wq, bq, Wk, bk, Wv, bv, Wo, bo, proj):
    sh = x.shape

    def heads(t):
        return jnp.swapaxes(t.reshape(sh[:-1] + (H, DH)), -2, -3)

    q = heads(x @ Wq + bq)
    k = heads(x @ Wk + bk)
    v = heads(x @ Wv + bv)
    dn = DH ** -0.25
    q = q * dn
    k = k * dn
    qd = q @ proj
    kd = k @ proj
    qn = 0.5 * jnp.sum(q * q, -1, keepdims=True)
    kn = 0.5 * jnp.sum(k * k, -1, keepdims=True)
    qp = jnp.exp(qd - qn - jnp.max(qd, -1, keepdims=True)) * (M ** -0.5) + 1e-6
    kp = jnp.exp(kd - kn - jnp.max(kd, axis=(-1, -2), keepdims=True)) * (M ** -0.5) + 1e-6
    kv = jnp.einsum('...lm,...ld->...md', kp, v)
    z = 1.0 / (jnp.einsum('...lm,...m->...l', qp, jnp.sum(kp, -2)) + 1e-6)
    o = jnp.einsum('...lm,...md->...ld', qp, kv) * z[..., None]
    o = jnp.swapaxes(o, -2, -3).reshape(sh)
    return o @ Wo + bo


def _moe(x, Wg, W1, b1, W2, b2):
    # top-2 softmax gate with renormalized weights, top_k-free formulation:
    # g_e = 1[l_e >= v2] * exp(l_e - v1) / (1 + exp(v2 - v1))
    logits = x @ Wg
    v1 = jnp.max(logits, -1, keepdims=True)
    masked = jnp.where(logits >= v1, -jnp.inf, logits)
    v2 = jnp.max(masked, -1, keepdims=True)
    sel = (logits >= v2).astype(x.dtype)
    g = sel * jnp.exp(logits - v1) / (1.0 + jnp.exp(v2 - v1))
    h = jax.nn.relu(jnp.einsum('...d,edf->...ef', x, W1) + b1)
    y = jnp.einsum('...ef,efd->...ed', h, W2) + b2
    return jnp.einsum('...ed,...e->...d', y, g)


def _block(x, Wq, bq, Wk, bk, Wv, bv, Wo, bo, proj, Wg, W1, b1, W2, b2,
           g2, be2, g3, be3):
    a = _performer(x, Wq, bq, Wk, bk, Wv, bv, Wo, bo, proj)
    x = _layernorm(x + a, g2, be2)
    f = _moe(x, Wg, W1, b1, W2, b2)
    return _layernorm(x + f, g3, be3)


def _forward_shard(n0, adaptive_sh, x_enc, x_mark, emb_W, emb_b, tod_table,
                   dow_table, sp_W, sp_Wq, sp_bq, sp_Wk, sp_bk, sp_Wv, sp_bv,
                   sp_Wo, sp_bo, sp_proj, sp_gate, sp_W1, sp_b1, sp_W2, sp_b2,
                   sp_ln2_g, sp_ln2_b, sp_ln3_g, sp_ln3_b, proj_W, proj_b,
                   LWq, Lbq, LWk, Lbk, LWv, Lbv, LWo, Lbo, Lproj, Lgate,
                   LW1, Lb1, LW2, Lb2, Lln2g, Lln2b, Lln3g, Lln3b,
                   mix_W, mix_b):
    # RevIN stats over the FULL x_enc (global over L and N)
    mean = jnp.mean(x_enc, axis=(1, 2), keepdims=True)
    std = jnp.sqrt(jnp.var(x_enc, axis=(1, 2), keepdims=True) + EPS)
    # shard slice over N
    x_sh = jax.lax.dynamic_slice_in_dim(x_enc, n0, NSH, axis=2)  # [B,L,NSH,C]
    xn = (x_sh - mean) / std
    xp = jnp.pad(xn, ((0, 0), (1, 1), (0, 0), (0, 0)))
    trend = (xp[:, :-2] + xp[:, 1:-1] + xp[:, 2:]) / 3.0
    season = xn - trend
    sp = jnp.swapaxes(xn, 1, 2).reshape(B, NSH, L * C) @ sp_W
    sp = _block(sp[:, :, None, :], sp_Wq, sp_bq, sp_Wk, sp_bk, sp_Wv, sp_bv,
                sp_Wo, sp_bo, sp_proj, sp_gate, sp_W1, sp_b1, sp_W2, sp_b2,
                sp_ln2_g, sp_ln2_b, sp_ln3_g, sp_ln3_b)
    tod_f = jnp.broadcast_to(x_mark[:, :, None, 6:7], (B, L, NSH, 1))
    dow_f = jnp.broadcast_to(x_mark[:, :, None, 2:3], (B, L, NSH, 1))
    te = jnp.swapaxes(jnp.concatenate([trend, tod_f, dow_f], -1) @ emb_W + emb_b, 1, 2)
    se = jnp.swapaxes(jnp.concatenate([season, tod_f, dow_f], -1) @ emb_W + emb_b, 1, 2)
    ad = jnp.broadcast_to(adaptive_sh.reshape(1, NSH, L, D), (B, NSH, L, D))
    tod_idx = (x_mark[..., 6] * TOD).astype(jnp.int32)
    tod_e = jnp.broadcast_to(tod_table[tod_idx][:, None], (B, NSH, L, D))
    dow_idx = x_mark[..., 2].astype(jnp.int32)
    dow_e = jnp.broadcast_to(dow_table[dow_idx][:, None], (B, NSH, L, D))
    spb = jnp.broadcast_to(sp, (B, NSH, L, D))
    xe = jnp.concatenate([te, se, ad, tod_e, dow_e, spb], -1) @ proj_W + proj_b
    for i in range(LAYERS):
        xe = _block(xe, LWq[i], Lbq[i], LWk[i], Lbk[i], LWv[i], Lbv[i],
                    LWo[i], Lbo[i], Lproj[i], Lgate[i], LW1[i], Lb1[i],
                    LW2[i], Lb2[i], Lln2g[i], Lln2b[i], Lln3g[i], Lln3b[i])
    out = xe.reshape(B, NSH, L * D) @ mix_W + mix_b
    out = jnp.swapaxes(out.reshape(B, NSH, PRED, C), 1, 2)  # [B,PRED,NSH,C]
    return out * std + mean


_WEIGHT_KEYS = ['x_enc', 'x_mark', 'emb_W', 'emb_b', 'tod_table', 'dow_table',
                'sp_W', 'sp_Wq', 'sp_bq', 'sp_Wk', 'sp_bk', 'sp_Wv', 'sp_bv',
                'sp_Wo', 'sp_bo', 'sp_proj', 'sp_gate', 'sp_W1', 'sp_b1',
                'sp_W2', 'sp_b2', 'sp_ln2_g', 'sp_ln2_b', 'sp_ln3_g',
                'sp_ln3_b', 'proj_W', 'proj_b', 'LWq', 'Lbq', 'LWk', 'Lbk',
                'LWv', 'Lbv', 'LWo', 'Lbo', 'Lproj', 'Lgate', 'LW1', 'Lb1',
                'LW2', 'Lb2', 'Lln2g', 'Lln2b', 'Lln3g', 'Lln3b',
                'mix_W', 'mix_b']

_pmapped = None


def _get_pmapped():
    global _pmapped
    if _pmapped is None:
        _pmapped = jax.pmap(
            _forward_shard,
            in_axes=(0, 0) + (None,) * len(_WEIGHT_KEYS),
            devices=jax.devices()[:NCORES],
        )
    return _pmapped


def kernel(**inputs):
    d = {k: np.asarray(v) for k, v in inputs.items()}
    # host-side sharding only: slice adaptive_table per core over N
    ad_full = d['adaptive_table'].reshape(N, L, D)
    ad_sh = np.stack([ad_full[c * NSH:(c + 1) * NSH] for c in range(NCORES)])
    n0 = np.arange(NCORES, dtype=np.int32) * NSH
    fn = _get_pmapped()
    args = [d[k] for k in _WEIGHT_KEYS]
    out_sh = fn(n0, ad_sh, *args)  # [8, B, PRED, NSH, C]
    out_sh = np.asarray(out_sh)
    out = np.concatenate([out_sh[c] for c in range(NCORES)], axis=2)
    return out.astype(np.float32)


if __name__ == '__main__':
    print(jax.devices())
